# revision 20
# baseline (speedup 1.0000x reference)
"""Trainium2 Bass kernel for nn_DistanceDecoder (moe_routing).

reference:
    comp_b  = components[object_labels]            # [B, 32, 6144]
    mean_b  = means[object_labels]                 # [B, 6144]
    out     = einsum('bp,bpo->bo', lattent, comp_b) + mean_b

Strategy (8 NeuronCores):
  * Shard OUT_DIM (6144) 8-ways -> each core owns a 768-wide column slice
    and the full batch.  Per-core HBM traffic is then ~4.3 MB (its own
    slice of the PCA table + 3 MB output) instead of the 18 MB the
    batch-parallel split would need (full 15 MB table replicated).
  * On the host, stable-sort the batch by label (MoE dispatch) and append
    the per-object mean as a 33rd row of each object's [32, 768] component
    block with a matching constant-1.0 row in the latent matrix, so gather
    + vecmat + mean-add is a single block-banded matmul
        out_T[768, 1024] = C2aug^T @ Epack
    over 7 K-tiles of 3 objects (K = 3*33 = 99 rows).  After the sort,
    each K-tile's samples form one contiguous column range, baked into the
    instruction stream as matmul free-dim offsets.
  * Matmul operands are fp16 by default: ~10-bit-mantissa rounding (same
    error class as the fp32r PE mode measured on HW) but full-rate
    pipelined matmuls and half the input DMA bytes.  DD_DTYPE=f32r swaps
    in fp32r (full fp32 operand bits); fp32r matmuls then require even
    range starts/widths, fixed by zero pad columns, with samples pushed
    past column 1024 computed on the host.
  * All input DMAs are issued before the first output DMA so later
    component chunks are never head-of-line blocked behind output
    transfers in the DMA queues.
  * Host applies the inverse permutation / column concat at the end.
"""

import os

import numpy as np

BATCH = 1024
PCA = 32
ROWS = PCA + 1             # 32 components + 1 mean row per object
OUT_DIM = 6144
NOBJ = 20
NCORES = 8
SLICE = OUT_DIM // NCORES  # 768
NCHUNK = SLICE // 128      # 6 chunks of 128 output rows (out_T partitions)
OBJ_PER_KT = 3             # objects per K-tile -> K = 3*33 = 99 <= 128
KTILES = (NOBJ + OBJ_PER_KT - 1) // OBJ_PER_KT  # 7
KP = OBJ_PER_KT * ROWS     # 99 partitions per K-tile
SEGS = [(0, 512), (512, 1024)]  # PSUM bank segments

DTYPE = os.environ.get("DD_DTYPE", "fp16")  # "fp16" | "f32r"

_NC_CACHE: dict = {}


def _kheight(t: int) -> int:
    return (min(OBJ_PER_KT * (t + 1), NOBJ) - OBJ_PER_KT * t) * ROWS


def _np_dtype():
    return np.float16 if DTYPE == "fp16" else np.float32


def _build_nc(ranges: tuple):
    """Build + compile the single-core Bass program (SPMD across 8 cores).

    ranges: KTILES+1 ints; ranges[t]..ranges[t+1] is the sorted-batch column
    range whose labels fall in objects [3t, 3t+3) — baked into the
    instruction stream as matmul free-dim offsets.
    """
    import concourse.mybir as mybir
    from concourse import bacc
    from concourse.tile import TileContext

    dt_in = mybir.dt.float16 if DTYPE == "fp16" else mybir.dt.float32r
    f32 = mybir.dt.float32
    f16 = mybir.dt.float16

    nc = bacc.Bacc("TRN2", target_bir_lowering=False, debug=False)

    # chunk-major comp layout, K-partitions zero-padded 99->128: SDMA
    # engines serve fixed partition groups of 8, so a 99-row DMA engages
    # only 12.4 of 16 engines; 128-row DMAs hit the full engine set
    # (observed 350-400 B/ns vs 210 for 99-row transfers).
    CCOLS = KTILES * 128
    KPP = 128
    comp_d = nc.dram_tensor(
        "comp", [NCHUNK * KPP, CCOLS], dt_in, kind="ExternalInput"
    )
    epack_d = nc.dram_tensor("epack", [KPP, BATCH], dt_in, kind="ExternalInput")
    out_d = nc.dram_tensor("out", [SLICE, BATCH], f16, kind="ExternalOutput")

    with TileContext(nc) as tc:
        with (
            tc.tile_pool(name="const", bufs=1) as cpool,
            tc.tile_pool(name="outp", bufs=6) as opool,
            tc.tile_pool(name="ps", bufs=8, space="PSUM") as pspool,
        ):
            # Dummy matmul chain on a memset tile: keeps the PE's HAM
            # activity window hot during the ~4.5us wait for the first
            # input DMA, so the real matmul stream below runs at the warm
            # 2.4 GHz clock instead of 1.2 GHz throughout.
            warm = cpool.tile([128, 512], dt_in)
            nc.gpsimd.memset(warm, 0.0)
            wps = pspool.tile([128, 512], f32, tag="ps", name="warm_ps")
            NWARM = 5
            for i in range(NWARM):
                nc.tensor.matmul(
                    wps,
                    warm[:, 0:128],
                    warm,
                    start=(i == 0),
                    stop=(i == NWARM - 1),
                )

            # Per-chunk input DMAs so chunk-j matmuls wait only on their own
            # 0.18MB slice (whole-tile deps stalled the stream 1.5-1.7us per
            # chunk group); first chunk + latents first so compute starts
            # as early as possible.
            # epack + last comp chunk ride the (otherwise idle-until-copies)
            # ACT HWDGE ring so they stream in parallel with the SP-ring
            # comp chunks instead of serializing ahead of them
            comps = []
            comps.append(cpool.tile([KPP, CCOLS], dt_in, name="comp0"))
            nc.sync.dma_start(out=comps[0], in_=comp_d[0:KPP, :])
            epack = cpool.tile([KPP, BATCH], dt_in)
            nc.scalar.dma_start(out=epack, in_=epack_d[:, :])
            for j in range(1, NCHUNK):
                cj = cpool.tile([KPP, CCOLS], dt_in, name=f"comp{j}")
                eng = nc.scalar if j == NCHUNK - 1 else nc.sync
                eng.dma_start(
                    out=cj, in_=comp_d[j * KPP : (j + 1) * KPP, :]
                )
                comps.append(cj)

            for j in range(NCHUNK):
                compj = comps[j]
                out_sb = opool.tile(
                    [128, BATCH], f16, tag="out_sb", name=f"osb{j}"
                )
                for h, (lo_h, hi_h) in enumerate(SEGS):
                    ps = pspool.tile([128, 512], f32, tag="ps", name=f"ps{j}_{h}")
                    pieces = []
                    for t in range(KTILES):
                        lo = max(ranges[t], lo_h)
                        hi = min(ranges[t + 1], hi_h)
                        if lo < hi:
                            pieces.append((t, lo, hi))
                    # disjoint column pieces cover the bank; first starts the
                    # accumulation group, later ones land on untouched
                    # elements (per-element has_written => plain writes)
                    for i, (t, lo, hi) in enumerate(pieces):
                        kh = _kheight(t)
                        nc.tensor.matmul(
                            ps[:, lo - lo_h : hi - lo_h],
                            compj[:kh, t * 128 : (t + 1) * 128],
                            epack[:kh, lo:hi],
                            start=(i == 0),
                            stop=(i == len(pieces) - 1),
                        )
                    # split the PSUM->SBUF drain (with f32->f16 cast) over
                    # both PSUM-capable engines; one alone trails the PE
                    if h == 0:
                        nc.vector.tensor_copy(out=out_sb[:, lo_h:hi_h], in_=ps)
                    else:
                        nc.scalar.copy(out_sb[:, lo_h:hi_h], ps)
                    if j == 0:
                        # chunk 0 stores per segment: starts the output
                        # stream ~1.1us earlier (right after the first CAST)
                        # while the Sync ring is otherwise idle
                        nc.sync.dma_start(
                            out=out_d[0:128, lo_h:hi_h],
                            in_=out_sb[:, lo_h:hi_h],
                        )
                # all output stores on the Sync ring: it is idle after
                # the input issues, while ACT-issued stores got scheduled
                # behind later copies (FIFO head-of-line, +2.6us)
                if j > 0:
                    nc.sync.dma_start(
                        out=out_d[j * 128 : (j + 1) * 128, :], in_=out_sb
                    )

    nc.compile()
    return nc


def _get_nc(ranges: tuple):
    if ranges not in _NC_CACHE:
        _NC_CACHE[ranges] = _build_nc(ranges)
    return _NC_CACHE[ranges]


def _prepare(lattent_codes, object_labels, means, components):
    x = np.ascontiguousarray(np.asarray(lattent_codes), dtype=np.float32)
    labels = np.asarray(object_labels).astype(np.int64)
    means = np.ascontiguousarray(np.asarray(means), dtype=np.float32)
    comp = np.ascontiguousarray(np.asarray(components), dtype=np.float32)
    ddt = _np_dtype()

    perm = np.argsort(labels, kind="stable")
    ls = labels[perm]
    xs = x[perm]  # [B, 32]

    counts = np.bincount(ls, minlength=NOBJ)
    cum = np.concatenate([[0], np.cumsum(counts)])
    raw = [int(cum[min(OBJ_PER_KT * t, NOBJ)]) for t in range(KTILES + 1)]
    widths = [raw[t + 1] - raw[t] for t in range(KTILES)]

    # fp32r matmuls need even range starts/widths -> insert zero pad columns
    # (dst_of_src maps sorted column -> padded column; samples pushed to
    # >= BATCH fall off the device batch and are computed on the host).
    # fp16 has no such ISA restriction: no padding at all.
    pad = (lambda w: w % 2) if DTYPE == "f32r" else (lambda w: 0)
    pstart = [0]
    for t in range(KTILES):
        pstart.append(pstart[t] + widths[t] + pad(widths[t]))
    ranges = tuple(min(p, BATCH) for p in pstart[:KTILES]) + (BATCH,)
    dst_of_src = np.concatenate(
        [np.arange(widths[t]) + pstart[t] for t in range(KTILES)]
    )
    on_dev = dst_of_src < BATCH

    # host-side fallback for overflow samples (at most a few, f32r only)
    ov = np.nonzero(~on_dev)[0]
    host_rows = None
    if len(ov):
        host_rows = (
            np.einsum("bp,bpo->bo", xs[ov], comp[ls[ov]]) + means[ls[ov]]
        ).astype(np.float32)

    # Epack[(l%3)*33 + p, dst(i)] = xs[i, p]; row (l%3)*33+32 = 1.0
    # (rows KP..127 stay zero: partition pad so DMAs engage all 16 SDMA
    # engines; matmuls only read the first kh<=99 partitions)
    band = (ls % OBJ_PER_KT).astype(np.int64)
    epack = np.zeros((128, BATCH), ddt)
    rows = band[None, on_dev] * ROWS + np.arange(PCA)[:, None]  # [32, n_dev]
    epack[rows, dst_of_src[None, on_dev]] = xs[on_dev].T.astype(ddt)
    epack[band[on_dev] * ROWS + PCA, dst_of_src[on_dev]] = 1.0

    # augmented component table: per object 32 component rows + 1 mean row
    m2 = np.concatenate([comp, means[:, None, :]], axis=1)  # [20, 33, OUT]
    m2 = m2.reshape(NOBJ * ROWS, OUT_DIM)

    in_maps = []
    CCOLS = KTILES * 128
    for c in range(NCORES):
        sl = slice(c * SLICE, (c + 1) * SLICE)
        arr = np.zeros((128, NCHUNK, KTILES, 128), ddt)
        for t in range(KTILES):
            kh = _kheight(t)
            blk = m2[KP * t : KP * t + kh, sl]  # [kh, 768]
            arr[:kh, :, t, :] = blk.reshape(kh, NCHUNK, 128).astype(ddt)
        # chunk-major with partition pad: [NCHUNK*128, CCOLS]; each chunk's
        # block is contiguous in DRAM and spans all 128 partitions
        comp_host = np.ascontiguousarray(
            arr.transpose(1, 0, 2, 3).reshape(NCHUNK * 128, CCOLS)
        )
        in_maps.append({"comp": comp_host, "epack": epack})
    return in_maps, ranges, perm, dst_of_src, on_dev, host_rows


def _assemble(results, perm, dst_of_src, on_dev, host_rows):
    out_s = np.empty((BATCH, OUT_DIM), np.float32)
    for c in range(NCORES):
        out_s[on_dev, c * SLICE : (c + 1) * SLICE] = results[c]["out"].T[
            dst_of_src[on_dev]
        ]
    if host_rows is not None:
        out_s[~on_dev] = host_rows
    out = np.empty_like(out_s)
    out[perm] = out_s
    return out


def run(inputs: dict, trace: bool = False):
    """Run on hardware; returns (full output, BassKernelResults)."""
    from concourse.bass_utils import run_bass_kernel_spmd

    in_maps, ranges, perm, dst_of_src, on_dev, host_rows = _prepare(**inputs)
    nc = _get_nc(ranges)
    res = run_bass_kernel_spmd(
        nc, in_maps, core_ids=list(range(NCORES)), trace=trace
    )
    return _assemble(res.results, perm, dst_of_src, on_dev, host_rows), res


def kernel(lattent_codes, object_labels, means, components) -> np.ndarray:
    out, _ = run(
        {
            "lattent_codes": lattent_codes,
            "object_labels": object_labels,
            "means": means,
            "components": components,
        }
    )
    return out



# revision 22
# speedup vs baseline: 1.0243x; 1.0243x over previous
"""Trainium2 Bass kernel for nn_DistanceDecoder (moe_routing).

reference:
    comp_b  = components[object_labels]            # [B, 32, 6144]
    mean_b  = means[object_labels]                 # [B, 6144]
    out     = einsum('bp,bpo->bo', lattent, comp_b) + mean_b

Strategy (8 NeuronCores):
  * Shard OUT_DIM (6144) 8-ways -> each core owns a 768-wide column slice
    and the full batch.  Per-core HBM traffic is then ~4.3 MB (its own
    slice of the PCA table + 3 MB output) instead of the 18 MB the
    batch-parallel split would need (full 15 MB table replicated).
  * On the host, stable-sort the batch by label (MoE dispatch) and append
    the per-object mean as a 33rd row of each object's [32, 768] component
    block with a matching constant-1.0 row in the latent matrix, so gather
    + vecmat + mean-add is a single block-banded matmul
        out_T[768, 1024] = C2aug^T @ Epack
    over 7 K-tiles of 3 objects (K = 3*33 = 99 rows).  After the sort,
    each K-tile's samples form one contiguous column range, baked into the
    instruction stream as matmul free-dim offsets.
  * Matmul operands are fp16 by default: ~10-bit-mantissa rounding (same
    error class as the fp32r PE mode measured on HW) but full-rate
    pipelined matmuls and half the input DMA bytes.  DD_DTYPE=f32r swaps
    in fp32r (full fp32 operand bits); fp32r matmuls then require even
    range starts/widths, fixed by zero pad columns, with samples pushed
    past column 1024 computed on the host.
  * Output is stored fp16 (PSUM->SBUF copies cast f32->f16) halving the
    3MB/core output DMA; host upcasts.  absmax rel err ~4.6e-4.
  * A short dummy-matmul chain on a memset tile runs while the first
    input DMAs are in flight, keeping the PE's HAM activity window hot so
    the real matmul stream runs at the warm 2.4 GHz clock (cold 1.2 GHz
    cost ~4us on this stream).
  * Inputs are zero-padded from 99 to 128 partitions: SDMA engines serve
    fixed 8-partition groups, so 128-row DMAs engage all 16 engines
    (~320 B/ns vs ~210 measured at 99 rows).  Comp is chunk-major so
    each per-chunk DMA is one contiguous DRAM block, and chunk-granular
    DMAs keep the matmul stream from stalling on whole-table transfers.
  * epack + the last comp chunk ride the ACT HWDGE ring, streaming in
    parallel with the SP-ring comp chunks; output stores go back on the
    SP ring (ACT-issued stores get scheduled behind later copies).
  * Host applies the inverse permutation / column concat at the end.

Measured on 8xTRN2 (exec_time includes a ~10.6us fixed NEFF tax: ~2.2us
preamble-to-first-DMA + ~8.4us postamble sem-clears/barrier that a
minimal 2-DMA kernel also pays): 26.7us baseline -> 22.1us.
"""

import os

import numpy as np

BATCH = 1024
PCA = 32
ROWS = PCA + 1             # 32 components + 1 mean row per object
OUT_DIM = 6144
NOBJ = 20
NCORES = 8
SLICE = OUT_DIM // NCORES  # 768
NCHUNK = SLICE // 128      # 6 chunks of 128 output rows (out_T partitions)
OBJ_PER_KT = 3             # objects per K-tile -> K = 3*33 = 99 <= 128
KTILES = (NOBJ + OBJ_PER_KT - 1) // OBJ_PER_KT  # 7
KP = OBJ_PER_KT * ROWS     # 99 partitions per K-tile
SEGS = [(0, 512), (512, 1024)]  # PSUM bank segments

DTYPE = os.environ.get("DD_DTYPE", "fp16")  # "fp16" | "f32r"

_NC_CACHE: dict = {}


def _kheight(t: int) -> int:
    return (min(OBJ_PER_KT * (t + 1), NOBJ) - OBJ_PER_KT * t) * ROWS


def _np_dtype():
    return np.float16 if DTYPE == "fp16" else np.float32


def _build_nc(ranges: tuple):
    """Build + compile the single-core Bass program (SPMD across 8 cores).

    ranges: KTILES+1 ints; ranges[t]..ranges[t+1] is the sorted-batch column
    range whose labels fall in objects [3t, 3t+3) — baked into the
    instruction stream as matmul free-dim offsets.
    """
    import concourse.mybir as mybir
    from concourse import bacc
    from concourse.tile import TileContext

    dt_in = mybir.dt.float16 if DTYPE == "fp16" else mybir.dt.float32r
    f32 = mybir.dt.float32
    f16 = mybir.dt.float16

    nc = bacc.Bacc("TRN2", target_bir_lowering=False, debug=False)

    # chunk-major comp layout, K-partitions zero-padded 99->128: SDMA
    # engines serve fixed partition groups of 8, so a 99-row DMA engages
    # only 12.4 of 16 engines; 128-row DMAs hit the full engine set
    # (observed 350-400 B/ns vs 210 for 99-row transfers).
    CCOLS = KTILES * 128
    KPP = 128
    comp_d = nc.dram_tensor(
        "comp", [NCHUNK * KPP, CCOLS], dt_in, kind="ExternalInput"
    )
    epack_d = nc.dram_tensor("epack", [KPP, BATCH], dt_in, kind="ExternalInput")
    out_d = nc.dram_tensor("out", [SLICE, BATCH], f16, kind="ExternalOutput")

    with TileContext(nc) as tc:
        with (
            tc.tile_pool(name="const", bufs=1) as cpool,
            tc.tile_pool(name="outp", bufs=6) as opool,
            tc.tile_pool(name="ps", bufs=8, space="PSUM") as pspool,
        ):
            # Dummy matmul chain on a memset tile: keeps the PE's HAM
            # activity window hot during the ~4.5us wait for the first
            # input DMA, so the real matmul stream below runs at the warm
            # 2.4 GHz clock instead of 1.2 GHz throughout.
            warm = cpool.tile([128, 512], dt_in)
            nc.gpsimd.memset(warm, 0.0)
            wps = pspool.tile([128, 512], f32, tag="ps", name="warm_ps")
            NWARM = 5
            for i in range(NWARM):
                nc.tensor.matmul(
                    wps,
                    warm[:, 0:128],
                    warm,
                    start=(i == 0),
                    stop=(i == NWARM - 1),
                )

            # Per-chunk input DMAs so chunk-j matmuls wait only on their own
            # 0.18MB slice (whole-tile deps stalled the stream 1.5-1.7us per
            # chunk group); first chunk + latents first so compute starts
            # as early as possible.
            # epack + last comp chunk ride the (otherwise idle-until-copies)
            # ACT HWDGE ring so they stream in parallel with the SP-ring
            # comp chunks instead of serializing ahead of them
            comps = []
            comps.append(cpool.tile([KPP, CCOLS], dt_in, name="comp0"))
            nc.sync.dma_start(out=comps[0], in_=comp_d[0:KPP, :])
            epack = cpool.tile([KPP, BATCH], dt_in)
            nc.scalar.dma_start(out=epack, in_=epack_d[:, :])
            for j in range(1, NCHUNK):
                cj = cpool.tile([KPP, CCOLS], dt_in, name=f"comp{j}")
                eng = nc.scalar if j == NCHUNK - 1 else nc.sync
                eng.dma_start(
                    out=cj, in_=comp_d[j * KPP : (j + 1) * KPP, :]
                )
                comps.append(cj)

            for j in range(NCHUNK):
                compj = comps[j]
                out_sb = opool.tile(
                    [128, BATCH], f16, tag="out_sb", name=f"osb{j}"
                )
                for h, (lo_h, hi_h) in enumerate(SEGS):
                    ps = pspool.tile([128, 512], f32, tag="ps", name=f"ps{j}_{h}")
                    pieces = []
                    for t in range(KTILES):
                        lo = max(ranges[t], lo_h)
                        hi = min(ranges[t + 1], hi_h)
                        if lo < hi:
                            pieces.append((t, lo, hi))
                    # disjoint column pieces cover the bank; first starts the
                    # accumulation group, later ones land on untouched
                    # elements (per-element has_written => plain writes)
                    for i, (t, lo, hi) in enumerate(pieces):
                        kh = _kheight(t)
                        nc.tensor.matmul(
                            ps[:, lo - lo_h : hi - lo_h],
                            compj[:kh, t * 128 : (t + 1) * 128],
                            epack[:kh, lo:hi],
                            start=(i == 0),
                            stop=(i == len(pieces) - 1),
                        )
                    # split the PSUM->SBUF drain (with f32->f16 cast) over
                    # both PSUM-capable engines; one alone trails the PE
                    if h == 0:
                        nc.vector.tensor_copy(out=out_sb[:, lo_h:hi_h], in_=ps)
                    else:
                        nc.scalar.copy(out_sb[:, lo_h:hi_h], ps)
                # all output stores on the Sync ring: it is idle after
                # the input issues, while ACT-issued stores got scheduled
                # behind later copies (FIFO head-of-line, +2.6us)
                nc.sync.dma_start(
                    out=out_d[j * 128 : (j + 1) * 128, :], in_=out_sb
                )

    nc.compile()
    return nc


def _get_nc(ranges: tuple):
    if ranges not in _NC_CACHE:
        _NC_CACHE[ranges] = _build_nc(ranges)
    return _NC_CACHE[ranges]


def _prepare(lattent_codes, object_labels, means, components):
    x = np.ascontiguousarray(np.asarray(lattent_codes), dtype=np.float32)
    labels = np.asarray(object_labels).astype(np.int64)
    means = np.ascontiguousarray(np.asarray(means), dtype=np.float32)
    comp = np.ascontiguousarray(np.asarray(components), dtype=np.float32)
    ddt = _np_dtype()

    perm = np.argsort(labels, kind="stable")
    ls = labels[perm]
    xs = x[perm]  # [B, 32]

    counts = np.bincount(ls, minlength=NOBJ)
    cum = np.concatenate([[0], np.cumsum(counts)])
    raw = [int(cum[min(OBJ_PER_KT * t, NOBJ)]) for t in range(KTILES + 1)]
    widths = [raw[t + 1] - raw[t] for t in range(KTILES)]

    # fp32r matmuls need even range starts/widths -> insert zero pad columns
    # (dst_of_src maps sorted column -> padded column; samples pushed to
    # >= BATCH fall off the device batch and are computed on the host).
    # fp16 has no such ISA restriction: no padding at all.
    pad = (lambda w: w % 2) if DTYPE == "f32r" else (lambda w: 0)
    pstart = [0]
    for t in range(KTILES):
        pstart.append(pstart[t] + widths[t] + pad(widths[t]))
    ranges = tuple(min(p, BATCH) for p in pstart[:KTILES]) + (BATCH,)
    dst_of_src = np.concatenate(
        [np.arange(widths[t]) + pstart[t] for t in range(KTILES)]
    )
    on_dev = dst_of_src < BATCH

    # host-side fallback for overflow samples (at most a few, f32r only)
    ov = np.nonzero(~on_dev)[0]
    host_rows = None
    if len(ov):
        host_rows = (
            np.einsum("bp,bpo->bo", xs[ov], comp[ls[ov]]) + means[ls[ov]]
        ).astype(np.float32)

    # Epack[(l%3)*33 + p, dst(i)] = xs[i, p]; row (l%3)*33+32 = 1.0
    # (rows KP..127 stay zero: partition pad so DMAs engage all 16 SDMA
    # engines; matmuls only read the first kh<=99 partitions)
    band = (ls % OBJ_PER_KT).astype(np.int64)
    epack = np.zeros((128, BATCH), ddt)
    rows = band[None, on_dev] * ROWS + np.arange(PCA)[:, None]  # [32, n_dev]
    epack[rows, dst_of_src[None, on_dev]] = xs[on_dev].T.astype(ddt)
    epack[band[on_dev] * ROWS + PCA, dst_of_src[on_dev]] = 1.0

    # augmented component table: per object 32 component rows + 1 mean row
    m2 = np.concatenate([comp, means[:, None, :]], axis=1)  # [20, 33, OUT]
    m2 = m2.reshape(NOBJ * ROWS, OUT_DIM)

    in_maps = []
    CCOLS = KTILES * 128
    for c in range(NCORES):
        sl = slice(c * SLICE, (c + 1) * SLICE)
        arr = np.zeros((128, NCHUNK, KTILES, 128), ddt)
        for t in range(KTILES):
            kh = _kheight(t)
            blk = m2[KP * t : KP * t + kh, sl]  # [kh, 768]
            arr[:kh, :, t, :] = blk.reshape(kh, NCHUNK, 128).astype(ddt)
        # chunk-major with partition pad: [NCHUNK*128, CCOLS]; each chunk's
        # block is contiguous in DRAM and spans all 128 partitions
        comp_host = np.ascontiguousarray(
            arr.transpose(1, 0, 2, 3).reshape(NCHUNK * 128, CCOLS)
        )
        in_maps.append({"comp": comp_host, "epack": epack})
    return in_maps, ranges, perm, dst_of_src, on_dev, host_rows


def _assemble(results, perm, dst_of_src, on_dev, host_rows):
    out_s = np.empty((BATCH, OUT_DIM), np.float32)
    for c in range(NCORES):
        out_s[on_dev, c * SLICE : (c + 1) * SLICE] = results[c]["out"].T[
            dst_of_src[on_dev]
        ]
    if host_rows is not None:
        out_s[~on_dev] = host_rows
    out = np.empty_like(out_s)
    out[perm] = out_s
    return out


def run(inputs: dict, trace: bool = False):
    """Run on hardware; returns (full output, BassKernelResults)."""
    from concourse.bass_utils import run_bass_kernel_spmd

    in_maps, ranges, perm, dst_of_src, on_dev, host_rows = _prepare(**inputs)
    nc = _get_nc(ranges)
    res = run_bass_kernel_spmd(
        nc, in_maps, core_ids=list(range(NCORES)), trace=trace
    )
    return _assemble(res.results, perm, dst_of_src, on_dev, host_rows), res


def kernel(lattent_codes, object_labels, means, components) -> np.ndarray:
    out, _ = run(
        {
            "lattent_codes": lattent_codes,
            "object_labels": object_labels,
            "means": means,
            "components": components,
        }
    )
    return out

